# revision 1
# baseline (speedup 1.0000x reference)
"""Trainium2 Bass kernel v2 for the GQA attention layer (B=2, S=2048, D=4096,
32 q-heads, 8 kv-heads, HD=128, RoPE, causal mask).

Sharding: 8 cores = 2 (batch) x 4 (head groups); column-parallel wq/wk/wv,
row-parallel wo; bf16 partial [S, D] outputs summed on host.

v2 changes vs baseline:
  - PE warmup preamble (zero matmuls, rep 0 only) spanning the initial DMA
    wait plus gap-filler matmuls during panel 0, so the HAM clock gate opens
    before real work and never re-throttles (startup PE gaps kept < 3.4 us).
  - panel j and attention block j interleaved (P0 A0 P1 A1 ...) so panel
    j+1's DMAs hide under attention block j's compute.
  - RoPE folded into the projection-PSUM evacuation (2 psum-reading muls +
    swap DMA + 1 add instead of copy + swap + 2 muls + add).
  - attention: score tiles processed in pairs sharing one 2-bank PSUM tile
    and ONE exp instruction (halves ScalarE fixed overhead); softmax
    denominator via GpSimd adds of prob tiles + one accumulated
    ones-matmul per 4-tile group (cuts the per-tile denominator matmul).
  - V projection packs 2 token-tiles per PSUM bank, single evacuation.
  - bf16 partial output (halves output DMA).
"""

import sys

if "/opt/trn_rl_repo" not in sys.path:
    sys.path.insert(0, "/opt/trn_rl_repo")

import math
from contextlib import ExitStack

import ml_dtypes
import numpy as np

import concourse.bass as bass  # noqa: F401
import concourse.tile as tile
from concourse import bacc, mybir
from concourse.bass_utils import run_bass_kernel_spmd

BF16 = ml_dtypes.bfloat16
F32 = mybir.dt.float32
BF = mybir.dt.bfloat16

B, S, D = 2, 2048, 4096
NH, NKV, HD = 32, 8, 128
G = 4
HPG = NH // G  # 8
KPG = NKV // G  # 2
SCALE = 1.0 / math.sqrt(HD)

NFT = D // 128  # 32 feature tiles
PTOK = 512
NPANEL = S // PTOK  # 4
NTT = S // 128  # 16
NSQ = S // 512  # 4
NOD = D // 512  # 8
WARMUP_MM = 26

_CACHE = {}


def _build_program(phases=(1, 2, 3), reps=1):
    nc = bacc.Bacc("TRN2", target_bir_lowering=False, debug=False, num_devices=8)

    xt = nc.dram_tensor("xt", [D, S], BF, kind="ExternalInput").ap()
    wq = nc.dram_tensor("wq", [HPG, 128, NFT * 128], BF, kind="ExternalInput").ap()
    wk = nc.dram_tensor("wk", [KPG, 128, NFT * 128], BF, kind="ExternalInput").ap()
    wv = nc.dram_tensor("wv", [128, NFT * KPG * 128], BF, kind="ExternalInput").ap()
    wo = nc.dram_tensor("wo", [NOD, 128, HPG * 512], BF, kind="ExternalInput").ap()
    cosb = nc.dram_tensor("cosb", [128, S], BF, kind="ExternalInput").ap()
    s2b = nc.dram_tensor("s2b", [128, S], BF, kind="ExternalInput").ap()
    diagm = nc.dram_tensor("diagm", [128, 128], F32, kind="ExternalInput").ap()
    ones = nc.dram_tensor("ones", [128, 128], BF, kind="ExternalInput").ap()
    outp = nc.dram_tensor("outp", [S, D], BF, kind="ExternalOutput").ap()

    EXP = mybir.ActivationFunctionType.Exp
    MULT = mybir.AluOpType.mult
    ADD = mybir.AluOpType.add

    with tile.TileContext(nc) as tc, ExitStack() as ctx:
        pool = lambda name, bufs: ctx.enter_context(tc.tile_pool(name=name, bufs=bufs))
        ppool = lambda name, bufs: ctx.enter_context(
            tc.tile_pool(name=name, bufs=bufs, space="PSUM")
        )

        persist = pool("persist", 1)
        xpool = pool("xpool", 5)
        wqpool = pool("wqpool", 2)
        ropepool = pool("ropepool", 4)
        probpool = pool("probpool", 4)
        gpool = pool("gpool", 3)
        bigden = pool("bigden", 2)
        wopool = pool("wopool", 2)
        outpool = pool("outpool", 3)
        warmpool = pool("warmpool", 1)

        ppP = ppool("ppP", 2)    # [128,512] f32 (1 bank): proj QK/V chains + oproj
        ppS = ppool("ppS", 2)    # [128,1024] f32 (2 banks): score pairs + warmup
        psAt = ppool("psAt", 1)  # [128,512] f32: attnV accum
        psD = ppool("psD", 1)    # [128,512] f32: denominators

        # ---- persistent tiles ----
        qt = [persist.tile([128, S], BF, tag=f"qt{h}", name=f"qt{h}") for h in range(HPG)]
        kt = [persist.tile([128, S], BF, tag=f"kt{k}", name=f"kt{k}") for k in range(KPG)]
        v_sb = persist.tile([128, NTT * KPG * 128], BF, tag="v", name="v_sb")
        v_w_sb = persist.tile([128, NFT * KPG * 128], BF, tag="vw", name="v_w_sb")
        at = [persist.tile([128, S], BF, tag=f"at{h}", name=f"at{h}") for h in range(HPG)]
        cos_sb = persist.tile([128, S], BF, tag="cos", name="cos_sb")
        s2_sb = persist.tile([128, S], BF, tag="s2", name="s2_sb")
        diag_sb = persist.tile([128, 128], F32, tag="diag", name="diag_sb")
        ones_sb = persist.tile([128, 128], BF, tag="ones", name="ones_sb")

        do1, do2, do3 = (1 in phases), (2 in phases), (3 in phases)
        xt_v = xt.rearrange("(f p) t -> p f t", p=128)

        # ---- PE warmup: zero matmuls spanning the first DMA wait ----
        wz = warmpool.tile([128, 640], BF, tag="wz", name="wz")
        nc.vector.memset(wz[:], 0.0)

        def warmup(n_mm=WARMUP_MM):
            for _ in range(n_mm):
                ps = ppS.tile([128, 1024], F32, tag="ppS", name="ps_warm")
                nc.tensor.matmul(
                    ps[:, 0:512], wz[:, 0:128], wz[:, 128:640],
                    start=True, stop=True,
                )

        def proj_panel(n, first=False):
            tok0 = n * PTOK
            halves = []
            wh0 = None
            for q4 in range(4):
                xq = xpool.tile([128, 8 * PTOK], BF, tag="xts", name="xq")
                nc.sync.dma_start(
                    xq.rearrange("p (f t) -> p f t", t=PTOK),
                    xt_v[:, q4 * 8 : (q4 + 1) * 8, tok0 : tok0 + PTOK],
                )
                halves.append((xq, q4 * 8))
                if first and q4 == 0:
                    # critical-path first: head-0 weights + rope tables land
                    # while the warmup matmuls run
                    wh0 = wqpool.tile([128, NFT * 128], BF, tag="wqt", name="wh")
                    nc.sync.dma_start(wh0[:], wq[0])
                    nc.sync.dma_start(cos_sb[:], cosb[:])
                    nc.sync.dma_start(s2_sb[:], s2b[:])

            qk_dst = list(qt) + list(kt)
            for hh in range(HPG + KPG):
                wsrc = wq[hh] if hh < HPG else wk[hh - HPG]
                if first and hh == 0:
                    wh = wh0
                else:
                    wh = wqpool.tile([128, NFT * 128], BF, tag="wqt", name="wh")
                    nc.sync.dma_start(wh[:], wsrc)
                if first and hh == 1:
                    warmup(12)
                ps = ppP.tile([128, PTOK], F32, tag="ppP", name="ps_qk")
                for xtile, f0 in halves:
                    for fl in range(8):
                        f = f0 + fl
                        nc.tensor.matmul(
                            ps[:],
                            wh[:, f * 128 : (f + 1) * 128],
                            xtile[:, fl * PTOK : (fl + 1) * PTOK],
                            start=(f == 0),
                            stop=(f == NFT - 1),
                        )
                    if first and hh <= 1:
                        # keep PE fed while the next x-quarter DMA lands
                        warmup(5)
                # fused evacuation + RoPE: dst = ps*cos + swap(ps*s2)
                dst = qk_dst[hh]
                u = ropepool.tile([128, PTOK], BF, tag="u", name="u")
                nc.vector.tensor_tensor(
                    u[:], ps[:], s2_sb[:, tok0 : tok0 + PTOK], MULT
                )
                nc.vector.tensor_tensor(
                    dst[:, tok0 : tok0 + PTOK],
                    ps[:],
                    cos_sb[:, tok0 : tok0 + PTOK],
                    MULT,
                )
                rsw = ropepool.tile([128, PTOK], BF, tag="rsw", name="rsw")
                nc.sync.dma_start(rsw[0:64, :], u[64:128, :])
                nc.sync.dma_start(rsw[64:128, :], u[0:64, :])
                nc.vector.tensor_tensor(
                    dst[:, tok0 : tok0 + PTOK],
                    dst[:, tok0 : tok0 + PTOK],
                    rsw[:],
                    ADD,
                )

            # V projection: 2 token-tiles packed per PSUM bank
            if n == 0:
                nc.sync.dma_start(v_w_sb[:], wv[:])
            for mp in range(PTOK // 256):
                ps = ppP.tile([128, 512], F32, tag="ppP", name="ps_v")
                for half in range(2):
                    m = mp * 2 + half
                    for xtile, f0 in halves:
                        for fl in range(8):
                            f = f0 + fl
                            nc.tensor.matmul(
                                ps[:, half * 256 : half * 256 + 256],
                                xtile[:, fl * PTOK + m * 128 : fl * PTOK + m * 128 + 128],
                                v_w_sb[:, f * 256 : (f + 1) * 256],
                                start=(f == 0),
                                stop=(f == NFT - 1),
                            )
                tglob = n * (PTOK // 128) + mp * 2
                nc.vector.tensor_copy(
                    v_sb[:, tglob * 256 : tglob * 256 + 512], ps[:]
                )

        def attn_block(j):
            sq0 = j * 512
            n_sk = 4 * (j + 1)
            n_pairs = n_sk // 2
            n_groups = n_sk // 4
            SKEWP = 2

            def off_of(t):
                r = t - 4 * j
                return 128 * r if r >= 0 else 0

            for h in range(HPG):
                kv = h // (HPG // KPG)
                ps_a = psAt.tile([128, 512], F32, tag="psAt", name="psAt_t")
                ps_d = psD.tile([128, 512], F32, tag="psD", name="psD_t")
                pts = {}
                group_acc = None
                group_first_off = 0
                group_idx = 0
                for uu in range(n_pairs + SKEWP):
                    if uu < n_pairs:
                        t0, t1 = 2 * uu, 2 * uu + 1
                        o0, o1 = off_of(t0), off_of(t1)
                        ps_s = ppS.tile([128, 1024], F32, tag="ppS", name="ps_s")
                        nc.tensor.matmul(
                            ps_s[:, o0:512],
                            kt[kv][:, t0 * 128 : (t0 + 1) * 128],
                            qt[h][:, sq0 + o0 : sq0 + 512],
                            start=True, stop=True,
                        )
                        nc.tensor.matmul(
                            ps_s[:, 512 + o1 : 1024],
                            kt[kv][:, t1 * 128 : (t1 + 1) * 128],
                            qt[h][:, sq0 + o1 : sq0 + 512],
                            start=True, stop=True,
                        )
                        if t0 - 4 * j >= 0:
                            nc.vector.tensor_add(
                                ps_s[:, o0 : o0 + 128],
                                ps_s[:, o0 : o0 + 128],
                                diag_sb[:],
                            )
                        if t1 - 4 * j >= 0:
                            nc.vector.tensor_add(
                                ps_s[:, 512 + o1 : 512 + o1 + 128],
                                ps_s[:, 512 + o1 : 512 + o1 + 128],
                                diag_sb[:],
                            )
                        pt = probpool.tile([128, 1024], BF, tag="probs", name="probs_t")
                        nc.scalar.activation(
                            pt[:, o0:1024], ps_s[:, o0:1024], EXP, scale=SCALE
                        )
                        pts[uu] = (pt, o0, o1)
                    if uu >= SKEWP:
                        u = uu - SKEWP
                        t0, t1 = 2 * u, 2 * u + 1
                        pt, o0, o1 = pts.pop(u)
                        nc.tensor.matmul(
                            ps_a[:, o0:512],
                            v_sb[:, t0 * 256 + kv * 128 : t0 * 256 + kv * 128 + 128],
                            pt[:, o0:512],
                            start=(t0 == 0),
                            stop=False,
                        )
                        nc.tensor.matmul(
                            ps_a[:, o1:512],
                            v_sb[:, t1 * 256 + kv * 128 : t1 * 256 + kv * 128 + 128],
                            pt[:, 512 + o1 : 1024],
                            start=False,
                            stop=(t1 == n_sk - 1),
                        )
                        # denominator: GpSimd adds of prob tiles, grouped 4
                        if u % 2 == 0:
                            group_acc = gpool.tile([128, 512], BF, tag="gacc", name="gacc")
                            group_first_off = o0
                            nc.gpsimd.tensor_tensor(
                                group_acc[:, o1:512],
                                pt[:, o1:512],
                                pt[:, 512 + o1 : 1024],
                                ADD,
                            )
                            if o1 > o0:
                                nc.gpsimd.tensor_copy(
                                    group_acc[:, o0:o1], pt[:, o0:o1]
                                )
                        else:
                            nc.gpsimd.tensor_tensor(
                                group_acc[:, o0:512],
                                group_acc[:, o0:512],
                                pt[:, o0:512],
                                ADD,
                            )
                            nc.gpsimd.tensor_tensor(
                                group_acc[:, o1:512],
                                group_acc[:, o1:512],
                                pt[:, 512 + o1 : 1024],
                                ADD,
                            )
                            g_off = group_first_off
                            nc.tensor.matmul(
                                ps_d[:, g_off:512],
                                ones_sb[:],
                                group_acc[:, g_off:512],
                                start=(group_idx == 0),
                                stop=(group_idx == n_groups - 1),
                            )
                            group_idx += 1
                inv_b = bigden.tile([128, 512], F32, tag="inv_b", name="inv_b")
                nc.vector.reciprocal(inv_b[:], ps_d[:])
                nc.vector.tensor_tensor(
                    at[h][:, sq0 : sq0 + 512], ps_a[:], inv_b[:], MULT
                )

        def oproj_all():
            for d in range(NOD):
                wod = wopool.tile([128, HPG * 512], BF, tag="wot", name="wod")
                nc.sync.dma_start(wod[:], wo[d])
                for m in range(NTT):
                    ps = ppP.tile([128, 512], F32, tag="ppP", name="ps_o")
                    for h in range(HPG):
                        nc.tensor.matmul(
                            ps[:],
                            at[h][:, m * 128 : (m + 1) * 128],
                            wod[:, h * 512 : (h + 1) * 512],
                            start=(h == 0),
                            stop=(h == HPG - 1),
                        )
                    osb = outpool.tile([128, 512], BF, tag="osb", name="osb")
                    nc.vector.tensor_copy(osb[:], ps[:])
                    nc.sync.dma_start(
                        outp[m * 128 : (m + 1) * 128, d * 512 : (d + 1) * 512], osb[:]
                    )

        for _rep in range(reps):
            if _rep == 0:
                warmup()
            if do1:
                proj_panel(0, first=(_rep == 0))
                if _rep == 0:
                    nc.sync.dma_start(diag_sb[:], diagm[:])
                    nc.sync.dma_start(ones_sb[:], ones[:])
            for n in range(1, NPANEL):
                if do2:
                    attn_block(n - 1)
                if do1:
                    proj_panel(n)
            if do2:
                attn_block(NPANEL - 1)
            if do3:
                oproj_all()

    nc.compile()
    return nc


_SPLIT_PERM = np.concatenate([np.arange(0, HD, 2), np.arange(1, HD, 2)])


def _host_prep(x, freqs_cos, freqs_sin, mask, wq, wk, wv, wo):
    """Build per-core input maps (8 cores = 2 batches x 4 head groups)."""
    x = np.asarray(x, np.float32)
    wq = np.asarray(wq, np.float32)
    wk = np.asarray(wk, np.float32)
    wv = np.asarray(wv, np.float32)
    wo = np.asarray(wo, np.float32)
    freqs_cos = np.asarray(freqs_cos, np.float32)
    freqs_sin = np.asarray(freqs_sin, np.float32)
    mask = np.asarray(mask, np.float32)

    xts = [np.ascontiguousarray(x[b].T).astype(BF16) for b in range(B)]

    ct = freqs_cos.T  # [64, S]
    st = freqs_sin.T
    cosb = np.concatenate([ct, ct], axis=0).astype(BF16)
    # s2 = [s; -s]: dst = ps*cos + swap(ps*s2) reproduces [-s; s] swap form
    s2b = np.concatenate([st, -st], axis=0).astype(BF16)
    diagm = np.ascontiguousarray(
        mask[0:128, 0:128].T * math.sqrt(HD), dtype=np.float32
    )
    ones = np.ones((128, 128), BF16)

    per_g = []
    for g in range(G):
        wq_g = wq[:, g * HPG * HD : (g + 1) * HPG * HD].reshape(D, HPG, HD)
        wq_g = wq_g[:, :, _SPLIT_PERM]
        wq_g = np.ascontiguousarray(
            wq_g.reshape(NFT, 128, HPG, HD).transpose(2, 1, 0, 3).reshape(HPG, 128, NFT * 128)
        ).astype(BF16)

        wk_g = wk[:, g * KPG * HD : (g + 1) * KPG * HD].reshape(D, KPG, HD)
        wk_g = wk_g[:, :, _SPLIT_PERM]
        wk_g = np.ascontiguousarray(
            wk_g.reshape(NFT, 128, KPG, HD).transpose(2, 1, 0, 3).reshape(KPG, 128, NFT * 128)
        ).astype(BF16)

        wv_g = np.ascontiguousarray(
            wv[:, g * KPG * HD : (g + 1) * KPG * HD]
            .reshape(NFT, 128, KPG * 128)
            .transpose(1, 0, 2)
            .reshape(128, NFT * KPG * 128)
        ).astype(BF16)

        wo_g = wo[g * HPG * HD : (g + 1) * HPG * HD, :]
        wo_g = np.ascontiguousarray(
            wo_g.reshape(HPG, 128, NOD, 512).transpose(2, 1, 0, 3).reshape(NOD, 128, HPG * 512)
        ).astype(BF16)

        per_g.append((wq_g, wk_g, wv_g, wo_g))

    in_maps = []
    for core in range(8):
        b, g = divmod(core, G)
        wq_g, wk_g, wv_g, wo_g = per_g[g]
        in_maps.append(
            {
                "xt": xts[b],
                "wq": wq_g,
                "wk": wk_g,
                "wv": wv_g,
                "wo": wo_g,
                "cosb": cosb,
                "s2b": s2b,
                "diagm": diagm,
                "ones": ones,
            }
        )
    return in_maps


def get_program(phases=(1, 2, 3), reps=1):
    key = ("nc", tuple(phases), reps)
    if key not in _CACHE:
        _CACHE[key] = _build_program(phases, reps)
    return _CACHE[key]


def kernel(
    x, start_pos, freqs_cos, freqs_sin, mask, wq, wk, wv, wo, **_ignored
):
    nc = get_program()
    in_maps = _host_prep(x, freqs_cos, freqs_sin, mask, wq, wk, wv, wo)
    res = run_bass_kernel_spmd(nc, in_maps, core_ids=list(range(8)))
    partials = [res.results[c]["outp"].astype(np.float32) for c in range(8)]
    out = np.stack(
        [
            partials[b * G]
            + partials[b * G + 1]
            + partials[b * G + 2]
            + partials[b * G + 3]
            for b in range(B)
        ]
    ).astype(np.float32)
    return out

